# revision 2
# baseline (speedup 1.0000x reference)
"""Trainium2 Bass kernel for the 4-layer LSTM decoder (nn_Decoder).

Strategy: model-parallel over hidden units across 8 NeuronCores.
  - Core c owns hidden units [128c, 128c+128) of every layer (all 4 gates),
    so it holds a [512, 1024] slice of each U_w[l] / W_w[l] and a [64, 1024]
    slice of L_w, all resident in SBUF as bf16 (about 7.5 MB/core).
  - Per step, per layer: core computes its gate slice as
    out[hid=128, batch=256] (hidden on partitions), applies the LSTM cell
    elementwise, and AllGathers the bf16 hy chunk [128, 256] so every core
    has the full h for the next layer / next step.
  - The output projection y = h3 @ L^T is sharded over rows (64 per core);
    the host reassembles the full [steps, 256, 512] output.

All shapes hardcoded for h0/c0 [4, 256, 1024], U_w [4, 4096, 1024],
W_w [3, 4096, 1024], L_w [512, 1024], steps=128.
"""

import os
import sys

import numpy as np

for p in ("/opt/trn_rl_repo", "/opt/trn_rl_repo/concourse"):
    if p not in sys.path:
        sys.path.insert(0, p)

NLAYERS = 4
BSZ = 256
NHID = 1024
NOUT = 512
NCORES = 8
HC = NHID // NCORES  # 128 hidden units per core
OC = NOUT // NCORES  # 64 output rows per core
KCH = NHID // 128  # 8 contraction chunks
# gate order in the fused free axis: i, f, o, c  (sigmoid on first 3, tanh on c)
GATE_ORDER = (0, 1, 3, 2)  # indices into pytorch [i, f, g(c), o] blocks


def _build(steps: int):
    import concourse.bacc as bacc
    import concourse.bass as bass
    import concourse.mybir as mybir
    import concourse.tile as tile

    dt = mybir.dt
    AF = mybir.ActivationFunctionType

    nc = bacc.Bacc(
        "TRN2", target_bir_lowering=False, debug=False, num_devices=NCORES
    )

    # ---- kernel I/O (per-core contents supplied via in_maps) ----
    # ut[l, p, k*512 + g*128 + j] = U_w[l, gate_row(g, j), 128k + p]  (bf16)
    ut_d = nc.dram_tensor("ut", [NLAYERS, 128, KCH * 4 * 128], dt.bfloat16,
                          kind="ExternalInput")
    wt_d = nc.dram_tensor("wt", [NLAYERS - 1, 128, KCH * 4 * 128], dt.bfloat16,
                          kind="ExternalInput")
    # lt[p, k*64 + j] = L_w[64c + j, 128k + p]
    lt_d = nc.dram_tensor("lt", [128, KCH * OC], dt.bfloat16,
                          kind="ExternalInput")
    # full transposed h0 (same on all cores): [l, p, k*256 + b]
    h0t_d = nc.dram_tensor("h0t", [NLAYERS, 128, KCH * BSZ], dt.bfloat16,
                           kind="ExternalInput")
    # own c0 chunk: [l, p, b]
    c0t_d = nc.dram_tensor("c0t", [NLAYERS, 128, BSZ], dt.float32,
                           kind="ExternalInput")
    out_d = nc.dram_tensor("out", [steps, OC, BSZ], dt.float32,
                           kind="ExternalOutput")

    rg = [list(range(NCORES))]

    with tile.TileContext(nc) as tc:
        with (
            tc.tile_pool(name="wpool", bufs=1) as wpool,
            tc.tile_pool(name="hpool", bufs=1) as hpool,
            tc.tile_pool(name="cellpool", bufs=3) as cellpool,
            tc.tile_pool(name="opool", bufs=3) as opool,
            tc.tile_pool(name="gpsum", bufs=3, space="PSUM") as gpsum,
            tc.tile_pool(name="lpsum", bufs=2, space="PSUM") as lpsum,
            tc.tile_pool(name="agdram", bufs=4, space="DRAM") as agdram,
        )\
        :
            # ---- resident weights ----
            u_sb = []
            w_sb = []
            for l in range(NLAYERS):
                u_t = wpool.tile([128, KCH * 512], dt.bfloat16, name=f"u{l}")
                nc.sync.dma_start(u_t[:], ut_d[l])
                u_sb.append(u_t)
            for l in range(NLAYERS - 1):
                w_t = wpool.tile([128, KCH * 512], dt.bfloat16, name=f"w{l}")
                nc.sync.dma_start(w_t[:], wt_d[l])
                w_sb.append(w_t)
            l_sb = wpool.tile([128, KCH * OC], dt.bfloat16, name="lt")
            nc.sync.dma_start(l_sb[:], lt_d[:])

            # ---- state ----
            # hT[l][parity]: full transposed h for layer l, [128, 8*256] bf16
            hT = [
                [
                    hpool.tile([128, KCH * BSZ], dt.bfloat16, name=f"h{l}_{p}")
                    for p in range(2)
                ]
                for l in range(NLAYERS)
            ]
            cx = []
            for l in range(NLAYERS):
                c_t = hpool.tile([128, BSZ], dt.float32, name=f"c{l}")
                nc.sync.dma_start(c_t[:], c0t_d[l])
                cx.append(c_t)
            for l in range(NLAYERS):
                nc.sync.dma_start(hT[l][0][:], h0t_d[l])

            def proj(t, cur):
                """out[t] = L @ h3(t)  (own 64-row slice)."""
                lp = lpsum.tile([OC, BSZ], dt.float32, tag="lp")
                for k in range(KCH):
                    nc.tensor.matmul(
                        lp[:],
                        l_sb[:, k * OC:(k + 1) * OC],
                        hT[3][cur][:, k * BSZ:(k + 1) * BSZ],
                        start=(k == 0),
                        stop=(k == KCH - 1),
                    )
                lo = opool.tile([OC, BSZ], dt.float32, tag="lo")
                nc.vector.tensor_copy(lo[:], lp[:])
                nc.sync.dma_start(out_d[t], lo[:])

            def step(t):
                cur, nxt = t % 2, (t + 1) % 2
                for l in range(NLAYERS):
                    # gates psum [128 hid, 4*256]: free = gate(i,f,o,c) x batch
                    # 2 PSUM banks: gi 0,1 -> bank 0; gi 2,3 -> bank 1.
                    # start=True clears has_written for the WHOLE bank, so it
                    # may appear exactly once per bank (on its first matmul);
                    # every other matmul uses start=False (fresh elements are
                    # overwritten since their has_written bit is clear).
                    g = gpsum.tile([128, 4 * BSZ], dt.float32, tag="g")
                    n_acc = KCH if l == 0 else 2 * KCH
                    acc = 0
                    # recurrent part: U_l^T chunks vs h_l(t)
                    for k in range(KCH):
                        last = acc == n_acc - 1
                        for gi in range(4):
                            nc.tensor.matmul(
                                g[:, gi * BSZ:(gi + 1) * BSZ],
                                u_sb[l][:, k * 512 + gi * 128:
                                         k * 512 + gi * 128 + 128],
                                hT[l][cur][:, k * BSZ:(k + 1) * BSZ],
                                start=(acc == 0 and gi % 2 == 0),
                                stop=(last and gi % 2 == 1),
                            )
                        acc += 1
                    # input part: W_{l-1}^T chunks vs hy_{l-1}(t) (fresh)
                    if l > 0:
                        for k in range(KCH):
                            last = acc == n_acc - 1
                            for gi in range(4):
                                nc.tensor.matmul(
                                    g[:, gi * BSZ:(gi + 1) * BSZ],
                                    w_sb[l - 1][:, k * 512 + gi * 128:
                                                k * 512 + gi * 128 + 128],
                                    hT[l - 1][nxt][:, k * BSZ:(k + 1) * BSZ],
                                    start=False,
                                    stop=(last and gi % 2 == 1),
                                )
                            acc += 1

                    # ---- LSTM cell (all [128 hid, 256 batch]) ----
                    sg = cellpool.tile([128, 3 * BSZ], dt.float32, tag="sg")
                    nc.scalar.activation(sg[:], g[:, :3 * BSZ], AF.Sigmoid)
                    tg = cellpool.tile([128, BSZ], dt.float32, tag="tg")
                    nc.scalar.activation(tg[:], g[:, 3 * BSZ:], AF.Tanh)
                    t1 = cellpool.tile([128, BSZ], dt.float32, tag="t1")
                    nc.vector.tensor_mul(t1[:], sg[:, BSZ:2 * BSZ], cx[l][:])
                    t2 = cellpool.tile([128, BSZ], dt.float32, tag="t2")
                    nc.vector.tensor_mul(t2[:], sg[:, :BSZ], tg[:])
                    nc.vector.tensor_add(cx[l][:], t1[:], t2[:])
                    tcy = cellpool.tile([128, BSZ], dt.float32, tag="tcy")
                    nc.scalar.activation(tcy[:], cx[l][:], AF.Tanh)
                    hy = cellpool.tile([128, BSZ], dt.bfloat16, tag="hy")
                    nc.vector.tensor_mul(hy[:], sg[:, 2 * BSZ:], tcy[:])

                    # ---- allgather hy chunk -> full hT[l][nxt] ----
                    ag_in = agdram.tile([128, BSZ], dt.bfloat16, tag="agi")
                    nc.sync.dma_start(ag_in[:], hy[:])
                    ag_out = agdram.tile([NCORES * 128, BSZ], dt.bfloat16,
                                         tag="ago", addr_space="Shared")
                    nc.gpsimd.collective_compute(
                        "AllGather",
                        mybir.AluOpType.bypass,
                        replica_groups=rg,
                        ins=[ag_in[:].opt()],
                        outs=[ag_out[:].opt()],
                    )
                    nc.sync.dma_start(
                        hT[l][nxt][:].rearrange("p (k b) -> p k b", k=KCH),
                        ag_out[:].rearrange("(k p) b -> p k b", p=128),
                    )

            for t in range(steps):
                proj(t, t % 2)
                if t < steps - 1:
                    step(t)

    nc.compile()
    return nc


def _prep_inputs(h0, c0, U_w, W_w, L_w):
    """Per-core numpy input prep (transpose + bf16 + gate reorder)."""
    import ml_dtypes

    bf16 = ml_dtypes.bfloat16
    h0 = np.asarray(h0, np.float32)
    c0 = np.asarray(c0, np.float32)
    U_w = np.asarray(U_w, np.float32)
    W_w = np.asarray(W_w, np.float32)
    L_w = np.asarray(L_w, np.float32)

    # full transposed h0, bf16, same for all cores: [l, p, k*256 + b]
    # h0t[l, p, k*B + b] = h0[l, b, 128k + p]
    h0t = np.ascontiguousarray(
        h0.reshape(NLAYERS, BSZ, KCH, 128).transpose(0, 3, 2, 1)
    ).reshape(NLAYERS, 128, KCH * BSZ).astype(bf16)

    in_maps = []
    for c in range(NCORES):
        hs = slice(HC * c, HC * c + HC)

        def pack_rec(w):  # [4H, H] -> [128, KCH*4*128]
            # rows for this core's hidden units, gate order (i, f, o, c)
            blocks = [w[g * NHID + HC * c: g * NHID + HC * c + HC, :]
                      for g in GATE_ORDER]
            ws = np.stack(blocks, 0)  # [4, 128, 1024]
            # out[p, k*512 + g*128 + j] = ws[g, j, 128k + p]
            return np.ascontiguousarray(
                ws.reshape(4, HC, KCH, 128).transpose(3, 2, 0, 1)
            ).reshape(128, KCH * 4 * HC).astype(bf16)

        ut = np.stack([pack_rec(U_w[l]) for l in range(NLAYERS)], 0)
        wt = np.stack([pack_rec(W_w[l]) for l in range(NLAYERS - 1)], 0)
        # lt[p, k*64 + j] = L_w[64c + j, 128k + p]
        lslice = L_w[OC * c: OC * c + OC, :]  # [64, 1024]
        lt = np.ascontiguousarray(
            lslice.reshape(OC, KCH, 128).transpose(2, 1, 0)
        ).reshape(128, KCH * OC).astype(bf16)
        # c0 own chunk [l, p, b] = c0[l, b, 128c + p]
        c0t = np.ascontiguousarray(c0[:, :, hs].transpose(0, 2, 1))

        in_maps.append({
            "ut": ut, "wt": wt, "lt": lt,
            "h0t": h0t, "c0t": c0t,
        })
    return in_maps


_CACHE = {}


def kernel(h0, c0, U_w, W_w, L_w, steps):
    from concourse import bass_utils

    steps = int(steps)
    if steps not in _CACHE:
        _CACHE[steps] = _build(steps)
    nc = _CACHE[steps]

    in_maps = _prep_inputs(h0, c0, U_w, W_w, L_w)
    res = bass_utils.run_bass_kernel_spmd(
        nc, in_maps, core_ids=list(range(NCORES)),
        trace=bool(int(os.environ.get("K_TRACE", "0"))),
    )
    outs = [res.results[c]["out"] for c in range(NCORES)]  # [S, 64, 256] each
    full = np.stack(outs, 0)  # [8, S, 64, 256]
    full = full.transpose(1, 3, 0, 2).reshape(steps, BSZ, NOUT)
    return np.ascontiguousarray(full.astype(np.float32))


if __name__ == "__main__":
    steps = int(os.environ.get("K_STEPS", "4"))
    rng = np.random.default_rng(0)
    h0 = rng.standard_normal((NLAYERS, BSZ, NHID), np.float32)
    c0 = rng.standard_normal((NLAYERS, BSZ, NHID), np.float32)
    s = 1.0 / np.sqrt(NHID)
    U_w = rng.uniform(-s, s, (NLAYERS, 4 * NHID, NHID)).astype(np.float32)
    W_w = rng.uniform(-s, s, (NLAYERS - 1, 4 * NHID, NHID)).astype(np.float32)
    L_w = rng.uniform(-s, s, (NOUT, NHID)).astype(np.float32)
    out = kernel(h0, c0, U_w, W_w, L_w, steps)
    print("out", out.shape, out.dtype, float(np.abs(out).mean()))
